# revision 20
# baseline (speedup 1.0000x reference)
"""Trainium2 Bass kernel: multi-head attention (B=2, T=2048, C=2048, H=16, D=128).

Sharding: tensor-parallel over heads. 8 cores x 2 heads each.
  - W_qkv columns sliced per head-pair, W_proj rows sliced per head-pair.
  - Each core computes a partial output; host sums the 8 partials.

Per-core dataflow (no on-device transposes anywhere):
  xT [C, B*T] (host-pre-transposed) is the shared activation input, held
  resident per batch as a [128, 16, 2048] tile (per-ck half-token DMAs,
  2KB lines, ordered so rt=0's chunks land first).
  1) q/k proj:  lhsT = W block (stationary), rhs = xT chunk
                -> qT/kT in [d, tokens] layout (PSUM), RoPE applied on the
                way to SBUF.
  2) v proj:    lhsT = xT chunk (stationary), rhs = Wv -> v [tokens, d].
  3) attention (ScalarE exp is the pacing engine; PE runs just below it):
     - mm1 pair (h0,h1) -> one [128,1024] fp32 PSUM chunk (2 banks)
     - ONE exp per chunk on ScalarE ((1024+352)/1.2 ns)
     - softmax denominator: every e-chunk feeds a PE ones-chain that
       accumulates [1,512] rows col-group packed 4-per-bank (PSUM adds
       commute, so chain MMs ride directly behind mm2 and e-tiles free
       immediately; only the first chain MM of a qt-pair uses start=True
       since that clears the whole bank's has_written bits).
     - ONE batched DVE reciprocal per qt pair; recip rows staged to
       partition 0 via GpSimd-triggered DMAs + partition_broadcast;
       normalize in place.  All spread over the next qt's stream.
     - mm2 accumulates yT[d, qi]; y banks freed by an unnormalized copy.
  4) out proj:  4 o_ps copies assemble a [128,2048] staging tile,
                DMA'd with 4KB lines into a host-unpacked layout.
"""

import math

import numpy as np

N_CORES = 8
B, T, C = 2, 2048, 2048
N_HEAD, D = 16, 128
HPC = N_HEAD // N_CORES          # heads per core
JC = HPC * D                     # per-core slice width of qkv/proj dims

RT = 512                         # q tile (moving free dim) in attention
KB = 128                         # key block (contraction tile) in attention

PHASE_MARKS = []


def _build(Bp, Tp, Cp, hpc, d):
    """Build the per-core Bass graph."""
    PHASE_MARKS.clear()
    import concourse.bacc as bacc
    import concourse.tile as tile
    from concourse import mybir

    f32 = mybir.dt.float32
    f32r = mybir.dt.float32r
    bf16 = mybir.dt.bfloat16
    Exp = mybir.ActivationFunctionType.Exp
    Copy = mybir.ActivationFunctionType.Copy

    jc = hpc * d
    BT = Bp * Tp
    n_ck = Cp // 128             # contraction chunks for proj
    n_kb = Tp // KB              # key blocks per batch
    n_qt = Tp // RT              # query tiles per batch
    n_rb = Tp // 128             # row blocks for out proj
    n_ot = Cp // RT              # output column tiles
    scale = 1.0 / math.sqrt(d)

    nc = bacc.Bacc("TRN2", target_bir_lowering=False, debug=False)

    # host-packed layouts
    xTp = nc.declare_dram_parameter("xTp", [128, n_ck, BT], bf16,
                                    isOutput=False)
    wqkv = nc.declare_dram_parameter("wqkv", [128, n_ck, 3 * jc], bf16,
                                     isOutput=False)
    wp = nc.declare_dram_parameter("wp", [jc, Cp], bf16, isOutput=False)
    ones_d = nc.declare_dram_parameter("ones", [128, 128], f32r, isOutput=False)
    cosT = nc.declare_dram_parameter("cosT", [d, Tp], bf16, isOutput=False)
    sinT = nc.declare_dram_parameter("sinT", [d, Tp], bf16, isOutput=False)
    # out[p, rb, :] = full_out[rb * 128 + p, :]   (host unpacks)
    out = nc.declare_dram_parameter("out", [128, BT // 128, Cp], bf16,
                                    isOutput=True)

    with tile.TileContext(nc) as tc:
        with (
            nc.allow_low_precision(reason="bf16 paths validated against the "
                                   "fp32 reference"),
            tc.tile_pool(name="wpool", bufs=1) as wpool,
            tc.tile_pool(name="acts", bufs=1) as acts,
            tc.tile_pool(name="xpool", bufs=1) as xpool,
            tc.tile_pool(name="rope", bufs=2) as rope,
            tc.tile_pool(name="epool", bufs=8) as epool,
            tc.tile_pool(name="small", bufs=2) as small,
            tc.tile_pool(name="bcpool", bufs=3) as bcpool,
            tc.tile_pool(name="opool", bufs=10) as opool,
        ):
            # ---- resident weights / tables / first xT batch ----
            RP = 256
            TH = Tp // 2         # half-token DMA extent (2KB lines)
            w_all = wpool.tile([128, n_ck, 3 * jc], bf16, tag="w")
            xt_tiles = {0: xpool.tile([128, n_ck, Tp], bf16, tag="xt",
                                      name="xt_b0")}

            def load_xt(b, half):
                t0 = half * TH
                for ck in range(n_ck):
                    nc.sync.dma_start(
                        xt_tiles[b][:, ck, t0:t0 + TH],
                        xTp[:, ck, b * Tp + t0:b * Tp + t0 + TH])

            nc.sync.dma_start(w_all[:, 0:1, :], wqkv[:, 0:1, :])
            nc.sync.dma_start(xt_tiles[0][:, 0, 0:TH], xTp[:, 0, 0:TH])
            nc.sync.dma_start(w_all[:, 1:4, :], wqkv[:, 1:4, :])
            for ck in range(1, n_ck):
                nc.sync.dma_start(xt_tiles[0][:, ck, 0:TH],
                                  xTp[:, ck, 0:TH])
                if ck == 4:
                    nc.sync.dma_start(w_all[:, 4:10, :], wqkv[:, 4:10, :])
                if ck == 10:
                    nc.sync.dma_start(w_all[:, 10:16, :], wqkv[:, 10:16, :])
            load_xt(0, 1)
            cos_sb = wpool.tile([d, Tp], bf16, tag="cos")
            sin_sb = wpool.tile([d, Tp], bf16, tag="sin")
            nc.sync.dma_start(cos_sb, cosT[:])
            nc.sync.dma_start(sin_sb, sinT[:])
            ones_sb = wpool.tile([128, 1], f32r, tag="ones")
            nc.sync.dma_start(ones_sb, ones_d[:, 0:1])
            ones_bf = wpool.tile([128, 1], bf16, tag="ones_bf")
            nc.vector.tensor_copy(out=ones_bf, in_=ones_sb)
            wp_sb = wpool.tile([128, hpc, Cp], bf16, tag="wp")

            def wq(ck, h):
                return w_all[:, ck, h * d:(h + 1) * d]

            def wk(ck, h):
                return w_all[:, ck, jc + h * d:jc + (h + 1) * d]

            def wv(ck):
                return w_all[:, ck, 2 * jc:3 * jc]

            for b in range(Bp):
                xt_b = xt_tiles[b]
                qT_sb = acts.tile([128, hpc, Tp], bf16, tag="qT")
                kT_sb = acts.tile([128, hpc, Tp], bf16, tag="kT")
                v_sb = acts.tile([128, n_kb, jc], bf16, tag="v")
                yT_sb = acts.tile([128, hpc, Tp], bf16, tag="yT")

                # ================= qkv projection =================
                PHASE_MARKS.append((f"proj{b}", nc.next_id()))
                n_sub = RP // 128
                with tc.tile_pool(name="ps_proj", bufs=2, space="PSUM") as psp:
                    for rt in range(Tp // RP):
                        tsl = slice(rt * RP, (rt + 1) * RP)
                        q_ps = psp.tile([128, hpc * RP], f32, tag="qps")
                        k_ps = psp.tile([128, hpc * RP], f32, tag="kps")
                        v_ps = psp.tile([128, n_sub * jc], f32, tag="vps", bufs=1)
                        for ck in range(n_ck):
                            xt = xt_b[:, ck, tsl]
                            first = ck == 0
                            last = ck == n_ck - 1
                            for h in range(hpc):
                                nc.tensor.matmul(
                                    q_ps[:, h * RP:(h + 1) * RP],
                                    wq(ck, h), xt, start=(first and h == 0),
                                    stop=(last and h == hpc - 1),
                                    skip_group_check=True)
                                nc.tensor.matmul(
                                    k_ps[:, h * RP:(h + 1) * RP],
                                    wk(ck, h), xt, start=(first and h == 0),
                                    stop=(last and h == hpc - 1),
                                    skip_group_check=True)
                            for s in range(n_sub):
                                nc.tensor.matmul(
                                    v_ps[:, s * jc:(s + 1) * jc],
                                    xt[:, s * 128:(s + 1) * 128],
                                    wv(ck), start=(first and s == 0),
                                    stop=(last and s == n_sub - 1),
                                    skip_group_check=True)
                        # rope epilogue
                        hd = d // 2
                        for h in range(hpc):
                            for ps, dst in (
                                (q_ps[:, h * RP:(h + 1) * RP], qT_sb),
                                (k_ps[:, h * RP:(h + 1) * RP], kT_sb),
                            ):
                                t1 = rope.tile([d, RP], f32, tag="t1")
                                nc.vector.tensor_mul(t1, ps, cos_sb[:, tsl])
                                t2 = rope.tile([d, RP], f32, tag="t2")
                                nc.vector.tensor_mul(
                                    t2[0:hd], ps[hd:d], sin_sb[0:hd, tsl])
                                nc.vector.tensor_mul(
                                    t2[hd:d], ps[0:hd], sin_sb[hd:d, tsl])
                                nc.vector.tensor_add(dst[:, h, tsl], t1, t2)
                        for s in range(n_sub):
                            nc.scalar.activation(
                                v_sb[:, rt * n_sub + s, :],
                                v_ps[:, s * jc:(s + 1) * jc], Copy)

                # ================= attention =================
                PHASE_MARKS.append((f"attn{b}", nc.next_id()))
                if b == 0:
                    nc.sync.dma_start(
                        wp_sb, wp.rearrange("(h p) o -> p h o", p=128))
                with (
                    tc.tile_pool(name="ps_s", bufs=2, space="PSUM") as ps_s,
                    tc.tile_pool(name="ps_y", bufs=2, space="PSUM") as ps_y,
                    tc.tile_pool(name="ps_d", bufs=2, space="PSUM") as ps_d,
                ):
                    def mm1pair(qt, j):
                        qsl = slice(qt * RT, (qt + 1) * RT)
                        s_ps = ps_s.tile([128, hpc * RT], f32, tag="s",
                                         name=f"sps{qt}_{j}")
                        for h in range(hpc):
                            nc.tensor.matmul(
                                s_ps[:, h * RT:(h + 1) * RT],
                                kT_sb[:, h, j * KB:(j + 1) * KB],
                                qT_sb[:, h, qsl],
                                start=True, stop=True,
                                skip_group_check=True)
                        return s_ps

                    state = {}

                    def start_qt(qt):
                        ys = [ps_y.tile([d, RT], f32, tag="y",
                                        name=f"yps{qt}_{h}")
                              for h in range(hpc)]
                        return ys

                    def step(qt, j, s_ps, dsum_ps):
                        ys = state[qt]
                        e_j = epool.tile([128, hpc * RT], bf16, tag="e",
                                         name=f"e{qt}_{j}")
                        nc.scalar.activation(e_j, s_ps, Exp, scale=scale)
                        for h in range(hpc):
                            nc.tensor.matmul(
                                ys[h],
                                v_sb[:, j, h * d:(h + 1) * d],
                                e_j[:, h * RT:(h + 1) * RT],
                                start=(j == 0), stop=(j == n_kb - 1),
                                skip_group_check=True)
                        # denominator ones-chain: only the very first MM of a
                        # qt-pair clears the bank (start=True); all others
                        # overwrite-where-unset / accumulate-where-set.
                        for h in range(hpc):
                            g = 32 * ((qt % 2) * hpc + h)
                            nc.tensor.matmul(
                                dsum_ps[g:g + 1, :],
                                ones_bf, e_j[:, h * RT:(h + 1) * RT],
                                start=(j == 0),
                                stop=(j == n_kb - 1),
                                skip_group_check=True,
                                tile_position=(0, g))

                    def finish_qt(qt):
                        ys = state.pop(qt)
                        qsl = slice(qt * RT, (qt + 1) * RT)
                        for h in range(hpc):
                            nc.vector.tensor_copy(out=yT_sb[:, h, qsl],
                                                  in_=ys[h])

                    def finalize_ops(qt_pair, dsum_ps):
                        """Micro-op generator: recip, stage+broadcast,
                        normalize; interleaved into later streams."""
                        r_sb = small.tile([128, RT], f32, tag="recip",
                                          name=f"recip{qt_pair}")
                        nc.vector.reciprocal(r_sb[0:97, :], dsum_ps[0:97, :])
                        yield
                        pending = []
                        for qt in (2 * qt_pair, 2 * qt_pair + 1):
                            for h in range(hpc):
                                g = 32 * ((qt % 2) * hpc + h)
                                st = bcpool.tile([1, RT], f32, tag="stage",
                                                 name=f"st{qt}_{h}")
                                nc.gpsimd.dma_start(st[0:1, :],
                                                    r_sb[g:g + 1, :])
                                bc = bcpool.tile([128, RT], f32, tag="bc",
                                                 name=f"bc{qt}_{h}")
                                nc.gpsimd.partition_broadcast(
                                    out_ap=bc, in_ap=st[0:1, :])
                                pending.append((qt, h, bc))
                                yield
                                if len(pending) >= 2:
                                    _emit_norm(*pending.pop(0))
                                    yield
                        while pending:
                            _emit_norm(*pending.pop(0))
                            yield

                    def _emit_norm(qt, h, bc):
                        qsl = slice(qt * RT, (qt + 1) * RT)
                        nc.vector.tensor_mul(
                            yT_sb[:, h, qsl], yT_sb[:, h, qsl], bc)

                    def drain(gen):
                        if gen is not None:
                            for _ in gen:
                                pass

                    state[0] = start_qt(0)
                    pend = [(0, 0, mm1pair(0, 0)), (0, 1, mm1pair(0, 1))]
                    dsum_ps = None
                    fin_gen = None
                    for qt in range(n_qt):
                        if qt % 2 == 0:
                            dsum_ps = ps_d.tile([128, RT], f32, tag="dsum",
                                                name=f"dsum{qt // 2}")
                        for j in range(n_kb):
                            cqt, cj, s_ps = pend.pop(0)
                            assert (cqt, cj) == (qt, j)
                            step(qt, j, s_ps, dsum_ps)
                            if fin_gen is not None and j >= 2:
                                if next(fin_gen, StopIteration) is StopIteration:
                                    fin_gen = None
                            nj = j + 2
                            if nj < n_kb:
                                pend.append((qt, nj, mm1pair(qt, nj)))
                            elif qt + 1 < n_qt:
                                if nj - n_kb == 0:
                                    state[qt + 1] = start_qt(qt + 1)
                                pend.append((qt + 1, nj - n_kb,
                                             mm1pair(qt + 1, nj - n_kb)))
                        # prefetch next batch's xT mid-attention (the tile is
                        # free once this batch's proj finished)
                        if qt == 1 and b + 1 < Bp:
                            xt_tiles[b + 1] = xpool.tile(
                                [128, n_ck, Tp], bf16, tag="xt",
                                name=f"xt_b{b + 1}")
                            load_xt(b + 1, 0)
                            load_xt(b + 1, 1)
                        finish_qt(qt)
                        if qt % 2 == 1:
                            drain(fin_gen)
                            fin_gen = finalize_ops(qt // 2, dsum_ps)
                            next(fin_gen)   # recip emitted inside pool scope

                # ============== output projection ==============
                # per-[128,512] copy + immediate DMA (baseline pacing); the
                # last qt-pair's finalize interleaves with the first blocks
                PHASE_MARKS.append((f"outproj{b}", nc.next_id()))
                with tc.tile_pool(name="ps_o", bufs=2, space="PSUM") as ps_o:
                    for rb in range(n_rb):
                        for ot in range(n_ot):
                            o_ps = ps_o.tile([128, RT], f32, tag="ops")
                            for h in range(hpc):
                                nc.tensor.matmul(
                                    o_ps,
                                    yT_sb[:, h, rb * 128:(rb + 1) * 128],
                                    wp_sb[:, h, ot * RT:(ot + 1) * RT],
                                    start=(h == 0), stop=(h == hpc - 1))
                            o_sb = opool.tile([128, RT], bf16, tag="o")
                            if ot % 2 == 0:
                                nc.vector.tensor_copy(out=o_sb, in_=o_ps)
                            else:
                                nc.scalar.activation(o_sb, o_ps, Copy)
                            if fin_gen is not None:
                                if next(fin_gen, StopIteration) is StopIteration:
                                    fin_gen = None
                            nc.sync.dma_start(
                                out[:, b * (Tp // 128) + rb,
                                    ot * RT:(ot + 1) * RT],
                                o_sb)

    PHASE_MARKS.append(("tail", nc.next_id()))
    nc.compile()
    return nc


def _prep_in_maps(x, cos, sin, W_qkv, W_proj, n_cores, hpc, d):
    """Host-side shard prep: pure layout work (transpose / slice / pack)."""
    Bp, Tp, Cp = x.shape
    jc = hpc * d
    n_ck = Cp // 128
    import ml_dtypes
    xT = np.ascontiguousarray(x.reshape(Bp * Tp, Cp).T)
    xTp = np.ascontiguousarray(
        xT.reshape(n_ck, 128, Bp * Tp).transpose(1, 0, 2)
    ).astype(ml_dtypes.bfloat16)
    cosT = np.ascontiguousarray(cos.T).astype(ml_dtypes.bfloat16)
    sinT = np.ascontiguousarray(sin.T).copy()
    sinT[: d // 2] *= -1.0
    sinT = sinT.astype(ml_dtypes.bfloat16)
    in_maps = []
    for c in range(n_cores):
        j0, j1 = c * jc, (c + 1) * jc
        wcat = np.concatenate(
            [W_qkv[:, j0:j1], W_qkv[:, Cp + j0:Cp + j1],
             W_qkv[:, 2 * Cp + j0:2 * Cp + j1]], axis=1)
        wpk = np.ascontiguousarray(
            wcat.reshape(n_ck, 128, 3 * jc).transpose(1, 0, 2)
        ).astype(ml_dtypes.bfloat16)
        in_maps.append({
            "xTp": xTp,
            "wqkv": wpk,
            "wp": np.ascontiguousarray(W_proj[j0:j1, :]).astype(ml_dtypes.bfloat16),
            "ones": np.ones((128, 128), dtype=np.float32),
            "cosT": cosT,
            "sinT": sinT,
        })
    return in_maps


def _install_ntff_hook():
    """Enable NTFF profiling under axon when the boot image lacks the
    antenv.axon_hooks shim. Harmless if anything is missing."""
    import sys
    import types
    try:
        from antenv.axon_hooks import get_axon_ntff_profile_hook
        if get_axon_ntff_profile_hook() is not None:
            return
    except ImportError:
        pass
    try:
        sys.path.insert(0, "/root/.axon_site")
        from trn_agent_boot.trn_boot import _ntff_profile_via_ctypes

        hook = _ntff_profile_via_ctypes("/opt/axon/libaxon_pjrt.so")
        if hook is None:
            return
        mod = types.ModuleType("antenv.axon_hooks")
        mod.get_axon_ntff_profile_hook = lambda: hook
        mod.set_axon_ntff_profile_hook = lambda h: None
        import antenv
        antenv.axon_hooks = mod
        sys.modules["antenv.axon_hooks"] = mod
    except Exception:
        pass


def _run(x, cos, sin, W_qkv, W_proj, trace=False):
    from concourse.bass_utils import run_bass_kernel_spmd

    if trace:
        _install_ntff_hook()

    x = np.ascontiguousarray(x, dtype=np.float32)
    cos = np.ascontiguousarray(cos, dtype=np.float32)
    sin = np.ascontiguousarray(sin, dtype=np.float32)
    W_qkv = np.ascontiguousarray(W_qkv, dtype=np.float32)
    W_proj = np.ascontiguousarray(W_proj, dtype=np.float32)

    Bp, Tp, Cp = x.shape
    nc = _build(Bp, Tp, Cp, HPC, D)
    in_maps = _prep_in_maps(x, cos, sin, W_qkv, W_proj, N_CORES, HPC, D)
    res = run_bass_kernel_spmd(nc, in_maps, core_ids=list(range(N_CORES)),
                               trace=trace)
    acc = np.zeros((Bp * Tp, Cp), dtype=np.float32)
    for i in range(N_CORES):
        o = np.asarray(res.results[i]["out"], dtype=np.float32)
        acc += o.transpose(1, 0, 2).reshape(Bp * Tp, Cp)
    return acc.reshape(Bp, Tp, Cp), res


def kernel(x, cos, sin, W_qkv, W_proj):
    out, _ = _run(x, cos, sin, W_qkv, W_proj, trace=False)
    return out


# revision 21
# speedup vs baseline: 1.2466x; 1.2466x over previous
"""Trainium2 Bass kernel: multi-head attention (B=2, T=2048, C=2048, H=16, D=128).

Sharding: tensor-parallel over heads. 8 cores x 2 heads each.
  - W_qkv columns sliced per head-pair, W_proj rows sliced per head-pair.
  - Each core computes a partial output; host sums the 8 partials.

Per-core dataflow (no on-device transposes anywhere):
  xT [C, B*T] (host-pre-transposed) is the shared activation input, held
  resident per batch as a [128, 16, 2048] tile (per-ck half-token DMAs,
  2KB lines, ordered so rt=0's chunks land first).
  1) q/k proj:  lhsT = W block (stationary), rhs = xT chunk
                -> qT/kT in [d, tokens] layout (PSUM), RoPE applied on the
                way to SBUF.
  2) v proj:    lhsT = xT chunk (stationary), rhs = Wv -> v [tokens, d].
  3) attention (ScalarE exp is the pacing engine; PE runs just below it):
     - mm1 pair (h0,h1) -> one [128,1024] fp32 PSUM chunk (2 banks)
     - ONE exp per chunk on ScalarE ((1024+352)/1.2 ns)
     - softmax denominator: every e-chunk feeds a PE ones-chain that
       accumulates [1,512] rows col-group packed 4-per-bank (PSUM adds
       commute, so chain MMs ride directly behind mm2 and e-tiles free
       immediately; only the first chain MM of a qt-pair uses start=True
       since that clears the whole bank's has_written bits).
     - ONE batched DVE reciprocal per qt pair; recip rows staged to
       partition 0 via GpSimd-triggered DMAs + partition_broadcast;
       normalize in place.  All spread over the next qt's stream.
     - mm2 accumulates yT[d, qi]; y banks freed by an unnormalized copy.
  4) out proj:  4 o_ps copies assemble a [128,2048] staging tile,
                DMA'd with 4KB lines into a host-unpacked layout.
"""

import math

import numpy as np

N_CORES = 8
B, T, C = 2, 2048, 2048
N_HEAD, D = 16, 128
HPC = N_HEAD // N_CORES          # heads per core
JC = HPC * D                     # per-core slice width of qkv/proj dims

RT = 512                         # q tile (moving free dim) in attention
KB = 128                         # key block (contraction tile) in attention

PHASE_MARKS = []


def _build(Bp, Tp, Cp, hpc, d):
    """Build the per-core Bass graph."""
    PHASE_MARKS.clear()
    import concourse.bacc as bacc
    import concourse.tile as tile
    from concourse import mybir

    f32 = mybir.dt.float32
    f32r = mybir.dt.float32r
    bf16 = mybir.dt.bfloat16
    Exp = mybir.ActivationFunctionType.Exp
    Copy = mybir.ActivationFunctionType.Copy

    jc = hpc * d
    BT = Bp * Tp
    n_ck = Cp // 128             # contraction chunks for proj
    n_kb = Tp // KB              # key blocks per batch
    n_qt = Tp // RT              # query tiles per batch
    n_rb = Tp // 128             # row blocks for out proj
    n_ot = Cp // RT              # output column tiles
    scale = 1.0 / math.sqrt(d)

    nc = bacc.Bacc("TRN2", target_bir_lowering=False, debug=False)

    # host-packed layouts
    xTp = nc.declare_dram_parameter("xTp", [128, n_ck, BT], bf16,
                                    isOutput=False)
    wqkv = nc.declare_dram_parameter("wqkv", [128, n_ck, 3 * jc], bf16,
                                     isOutput=False)
    wp = nc.declare_dram_parameter("wp", [jc, Cp], bf16, isOutput=False)
    ones_d = nc.declare_dram_parameter("ones", [128, 128], f32r, isOutput=False)
    cosT = nc.declare_dram_parameter("cosT", [d, Tp], bf16, isOutput=False)
    sinT = nc.declare_dram_parameter("sinT", [d, Tp], bf16, isOutput=False)
    # out[p, rb, :] = full_out[rb * 128 + p, :]   (host unpacks)
    out = nc.declare_dram_parameter("out", [128, BT // 128, Cp], bf16,
                                    isOutput=True)

    with tile.TileContext(nc) as tc:
        with (
            nc.allow_low_precision(reason="bf16 paths validated against the "
                                   "fp32 reference"),
            tc.tile_pool(name="wpool", bufs=1) as wpool,
            tc.tile_pool(name="acts", bufs=1) as acts,
            tc.tile_pool(name="xpool", bufs=1) as xpool,
            tc.tile_pool(name="rope", bufs=2) as rope,
            tc.tile_pool(name="epool", bufs=8) as epool,
            tc.tile_pool(name="small", bufs=2) as small,
            tc.tile_pool(name="bcpool", bufs=3) as bcpool,
            tc.tile_pool(name="opool", bufs=3) as opool,
        ):
            # ---- resident weights / tables / first xT batch ----
            RP = 256
            TH = Tp // 2         # half-token DMA extent (2KB lines)
            w_all = wpool.tile([128, n_ck, 3 * jc], bf16, tag="w")
            xt_tiles = {0: xpool.tile([128, n_ck, Tp], bf16, tag="xt",
                                      name="xt_b0")}

            def load_xt(b, half):
                t0 = half * TH
                for ck in range(n_ck):
                    nc.sync.dma_start(
                        xt_tiles[b][:, ck, t0:t0 + TH],
                        xTp[:, ck, b * Tp + t0:b * Tp + t0 + TH])

            nc.sync.dma_start(w_all[:, 0:1, :], wqkv[:, 0:1, :])
            nc.sync.dma_start(xt_tiles[0][:, 0, 0:TH], xTp[:, 0, 0:TH])
            nc.sync.dma_start(w_all[:, 1:4, :], wqkv[:, 1:4, :])
            for ck in range(1, n_ck):
                nc.sync.dma_start(xt_tiles[0][:, ck, 0:TH],
                                  xTp[:, ck, 0:TH])
                if ck == 4:
                    nc.sync.dma_start(w_all[:, 4:10, :], wqkv[:, 4:10, :])
                if ck == 10:
                    nc.sync.dma_start(w_all[:, 10:16, :], wqkv[:, 10:16, :])
            load_xt(0, 1)
            cos_sb = wpool.tile([d, Tp], bf16, tag="cos")
            sin_sb = wpool.tile([d, Tp], bf16, tag="sin")
            nc.sync.dma_start(cos_sb, cosT[:])
            nc.sync.dma_start(sin_sb, sinT[:])
            ones_sb = wpool.tile([128, 1], f32r, tag="ones")
            nc.sync.dma_start(ones_sb, ones_d[:, 0:1])
            ones_bf = wpool.tile([128, 1], bf16, tag="ones_bf")
            nc.vector.tensor_copy(out=ones_bf, in_=ones_sb)
            wp_sb = wpool.tile([128, hpc, Cp], bf16, tag="wp")

            def wq(ck, h):
                return w_all[:, ck, h * d:(h + 1) * d]

            def wk(ck, h):
                return w_all[:, ck, jc + h * d:jc + (h + 1) * d]

            def wv(ck):
                return w_all[:, ck, 2 * jc:3 * jc]

            for b in range(Bp):
                xt_b = xt_tiles[b]
                qT_sb = acts.tile([128, hpc, Tp], bf16, tag="qT")
                kT_sb = acts.tile([128, hpc, Tp], bf16, tag="kT")
                v_sb = acts.tile([128, n_kb, jc], bf16, tag="v")
                yT_sb = acts.tile([128, hpc, Tp], bf16, tag="yT")

                # ================= qkv projection =================
                PHASE_MARKS.append((f"proj{b}", nc.next_id()))
                n_sub = RP // 128
                with tc.tile_pool(name="ps_proj", bufs=2, space="PSUM") as psp:
                    for rt in range(Tp // RP):
                        tsl = slice(rt * RP, (rt + 1) * RP)
                        q_ps = psp.tile([128, hpc * RP], f32, tag="qps")
                        k_ps = psp.tile([128, hpc * RP], f32, tag="kps")
                        v_ps = psp.tile([128, n_sub * jc], f32, tag="vps", bufs=1)
                        for ck in range(n_ck):
                            xt = xt_b[:, ck, tsl]
                            first = ck == 0
                            last = ck == n_ck - 1
                            for h in range(hpc):
                                nc.tensor.matmul(
                                    q_ps[:, h * RP:(h + 1) * RP],
                                    wq(ck, h), xt, start=(first and h == 0),
                                    stop=(last and h == hpc - 1),
                                    skip_group_check=True)
                                nc.tensor.matmul(
                                    k_ps[:, h * RP:(h + 1) * RP],
                                    wk(ck, h), xt, start=(first and h == 0),
                                    stop=(last and h == hpc - 1),
                                    skip_group_check=True)
                            for s in range(n_sub):
                                nc.tensor.matmul(
                                    v_ps[:, s * jc:(s + 1) * jc],
                                    xt[:, s * 128:(s + 1) * 128],
                                    wv(ck), start=(first and s == 0),
                                    stop=(last and s == n_sub - 1),
                                    skip_group_check=True)
                        # rope epilogue
                        hd = d // 2
                        for h in range(hpc):
                            for ps, dst in (
                                (q_ps[:, h * RP:(h + 1) * RP], qT_sb),
                                (k_ps[:, h * RP:(h + 1) * RP], kT_sb),
                            ):
                                t1 = rope.tile([d, RP], f32, tag="t1")
                                nc.vector.tensor_mul(t1, ps, cos_sb[:, tsl])
                                t2 = rope.tile([d, RP], f32, tag="t2")
                                nc.vector.tensor_mul(
                                    t2[0:hd], ps[hd:d], sin_sb[0:hd, tsl])
                                nc.vector.tensor_mul(
                                    t2[hd:d], ps[0:hd], sin_sb[hd:d, tsl])
                                nc.vector.tensor_add(dst[:, h, tsl], t1, t2)
                        for s in range(n_sub):
                            nc.scalar.activation(
                                v_sb[:, rt * n_sub + s, :],
                                v_ps[:, s * jc:(s + 1) * jc], Copy)

                # ================= attention =================
                PHASE_MARKS.append((f"attn{b}", nc.next_id()))
                if b == 0:
                    nc.sync.dma_start(
                        wp_sb, wp.rearrange("(h p) o -> p h o", p=128))
                with (
                    tc.tile_pool(name="ps_s", bufs=2, space="PSUM") as ps_s,
                    tc.tile_pool(name="ps_y", bufs=2, space="PSUM") as ps_y,
                    tc.tile_pool(name="ps_d", bufs=2, space="PSUM") as ps_d,
                ):
                    def mm1pair(qt, j):
                        qsl = slice(qt * RT, (qt + 1) * RT)
                        s_ps = ps_s.tile([128, hpc * RT], f32, tag="s",
                                         name=f"sps{qt}_{j}")
                        for h in range(hpc):
                            nc.tensor.matmul(
                                s_ps[:, h * RT:(h + 1) * RT],
                                kT_sb[:, h, j * KB:(j + 1) * KB],
                                qT_sb[:, h, qsl],
                                start=True, stop=True,
                                skip_group_check=True)
                        return s_ps

                    state = {}

                    def start_qt(qt):
                        ys = [ps_y.tile([d, RT], f32, tag="y",
                                        name=f"yps{qt}_{h}")
                              for h in range(hpc)]
                        return ys

                    def step(qt, j, s_ps, dsum_ps):
                        ys = state[qt]
                        e_j = epool.tile([128, hpc * RT], bf16, tag="e",
                                         name=f"e{qt}_{j}")
                        nc.scalar.activation(e_j, s_ps, Exp, scale=scale)
                        for h in range(hpc):
                            nc.tensor.matmul(
                                ys[h],
                                v_sb[:, j, h * d:(h + 1) * d],
                                e_j[:, h * RT:(h + 1) * RT],
                                start=(j == 0), stop=(j == n_kb - 1),
                                skip_group_check=True)
                        # denominator ones-chain: only the very first MM of a
                        # qt-pair clears the bank (start=True); all others
                        # overwrite-where-unset / accumulate-where-set.
                        for h in range(hpc):
                            g = 32 * ((qt % 2) * hpc + h)
                            nc.tensor.matmul(
                                dsum_ps[g:g + 1, :],
                                ones_bf, e_j[:, h * RT:(h + 1) * RT],
                                start=(j == 0),
                                stop=(j == n_kb - 1),
                                skip_group_check=True,
                                tile_position=(0, g))

                    def finish_qt(qt):
                        ys = state.pop(qt)
                        qsl = slice(qt * RT, (qt + 1) * RT)
                        for h in range(hpc):
                            nc.vector.tensor_copy(out=yT_sb[:, h, qsl],
                                                  in_=ys[h])

                    def finalize_ops(qt_pair, dsum_ps):
                        """Micro-op generator: recip, stage+broadcast,
                        normalize; interleaved into later streams."""
                        r_sb = small.tile([128, RT], f32, tag="recip",
                                          name=f"recip{qt_pair}")
                        nc.vector.reciprocal(r_sb[0:97, :], dsum_ps[0:97, :])
                        yield
                        pending = []
                        for qt in (2 * qt_pair, 2 * qt_pair + 1):
                            for h in range(hpc):
                                g = 32 * ((qt % 2) * hpc + h)
                                st = bcpool.tile([1, RT], f32, tag="stage",
                                                 name=f"st{qt}_{h}")
                                nc.gpsimd.dma_start(st[0:1, :],
                                                    r_sb[g:g + 1, :])
                                bc = bcpool.tile([128, RT], f32, tag="bc",
                                                 name=f"bc{qt}_{h}")
                                nc.gpsimd.partition_broadcast(
                                    out_ap=bc, in_ap=st[0:1, :])
                                pending.append((qt, h, bc))
                                yield
                                if len(pending) >= 2:
                                    _emit_norm(*pending.pop(0))
                                    yield
                        while pending:
                            _emit_norm(*pending.pop(0))
                            yield

                    def _emit_norm(qt, h, bc):
                        qsl = slice(qt * RT, (qt + 1) * RT)
                        nc.vector.tensor_mul(
                            yT_sb[:, h, qsl], yT_sb[:, h, qsl], bc)

                    def drain(gen):
                        if gen is not None:
                            for _ in gen:
                                pass

                    state[0] = start_qt(0)
                    pend = [(0, 0, mm1pair(0, 0)), (0, 1, mm1pair(0, 1))]
                    dsum_ps = None
                    fin_gen = None
                    for qt in range(n_qt):
                        if qt % 2 == 0:
                            dsum_ps = ps_d.tile([128, RT], f32, tag="dsum",
                                                name=f"dsum{qt // 2}")
                        for j in range(n_kb):
                            cqt, cj, s_ps = pend.pop(0)
                            assert (cqt, cj) == (qt, j)
                            step(qt, j, s_ps, dsum_ps)
                            if fin_gen is not None and j >= 2:
                                if next(fin_gen, StopIteration) is StopIteration:
                                    fin_gen = None
                            nj = j + 2
                            if nj < n_kb:
                                pend.append((qt, nj, mm1pair(qt, nj)))
                            elif qt + 1 < n_qt:
                                if nj - n_kb == 0:
                                    state[qt + 1] = start_qt(qt + 1)
                                pend.append((qt + 1, nj - n_kb,
                                             mm1pair(qt + 1, nj - n_kb)))
                        # prefetch next batch's xT mid-attention (the tile is
                        # free once this batch's proj finished)
                        if qt == 1 and b + 1 < Bp:
                            xt_tiles[b + 1] = xpool.tile(
                                [128, n_ck, Tp], bf16, tag="xt",
                                name=f"xt_b{b + 1}")
                            load_xt(b + 1, 0)
                            load_xt(b + 1, 1)
                        finish_qt(qt)
                        if qt % 2 == 1:
                            drain(fin_gen)
                            fin_gen = finalize_ops(qt // 2, dsum_ps)
                            next(fin_gen)   # recip emitted inside pool scope

                # ============== output projection ==============
                # wide [128, 2048] staging tiles -> 4KB-line DMAs; the last
                # qt-pair's finalize interleaves with the first rb blocks
                PHASE_MARKS.append((f"outproj{b}", nc.next_id()))
                with tc.tile_pool(name="ps_o", bufs=3, space="PSUM") as ps_o:
                    for rb in range(n_rb):
                        o_sb = opool.tile([128, Cp], bf16, tag="o")
                        for ot in range(n_ot):
                            o_ps = ps_o.tile([128, RT], f32, tag="ops")
                            for h in range(hpc):
                                nc.tensor.matmul(
                                    o_ps,
                                    yT_sb[:, h, rb * 128:(rb + 1) * 128],
                                    wp_sb[:, h, ot * RT:(ot + 1) * RT],
                                    start=(h == 0), stop=(h == hpc - 1))
                            osl = o_sb[:, ot * RT:(ot + 1) * RT]
                            if ot % 2 == 0:
                                nc.vector.tensor_copy(out=osl, in_=o_ps)
                            else:
                                nc.scalar.activation(osl, o_ps, Copy)
                            if fin_gen is not None:
                                if next(fin_gen, StopIteration) is StopIteration:
                                    fin_gen = None
                        nc.sync.dma_start(
                            out[:, b * (Tp // 128) + rb, :], o_sb)

    PHASE_MARKS.append(("tail", nc.next_id()))
    nc.compile()
    return nc


def _prep_in_maps(x, cos, sin, W_qkv, W_proj, n_cores, hpc, d):
    """Host-side shard prep: pure layout work (transpose / slice / pack)."""
    Bp, Tp, Cp = x.shape
    jc = hpc * d
    n_ck = Cp // 128
    import ml_dtypes
    xT = np.ascontiguousarray(x.reshape(Bp * Tp, Cp).T)
    xTp = np.ascontiguousarray(
        xT.reshape(n_ck, 128, Bp * Tp).transpose(1, 0, 2)
    ).astype(ml_dtypes.bfloat16)
    cosT = np.ascontiguousarray(cos.T).astype(ml_dtypes.bfloat16)
    sinT = np.ascontiguousarray(sin.T).copy()
    sinT[: d // 2] *= -1.0
    sinT = sinT.astype(ml_dtypes.bfloat16)
    in_maps = []
    for c in range(n_cores):
        j0, j1 = c * jc, (c + 1) * jc
        wcat = np.concatenate(
            [W_qkv[:, j0:j1], W_qkv[:, Cp + j0:Cp + j1],
             W_qkv[:, 2 * Cp + j0:2 * Cp + j1]], axis=1)
        wpk = np.ascontiguousarray(
            wcat.reshape(n_ck, 128, 3 * jc).transpose(1, 0, 2)
        ).astype(ml_dtypes.bfloat16)
        in_maps.append({
            "xTp": xTp,
            "wqkv": wpk,
            "wp": np.ascontiguousarray(W_proj[j0:j1, :]).astype(ml_dtypes.bfloat16),
            "ones": np.ones((128, 128), dtype=np.float32),
            "cosT": cosT,
            "sinT": sinT,
        })
    return in_maps


def _install_ntff_hook():
    """Enable NTFF profiling under axon when the boot image lacks the
    antenv.axon_hooks shim. Harmless if anything is missing."""
    import sys
    import types
    try:
        from antenv.axon_hooks import get_axon_ntff_profile_hook
        if get_axon_ntff_profile_hook() is not None:
            return
    except ImportError:
        pass
    try:
        sys.path.insert(0, "/root/.axon_site")
        from trn_agent_boot.trn_boot import _ntff_profile_via_ctypes

        hook = _ntff_profile_via_ctypes("/opt/axon/libaxon_pjrt.so")
        if hook is None:
            return
        mod = types.ModuleType("antenv.axon_hooks")
        mod.get_axon_ntff_profile_hook = lambda: hook
        mod.set_axon_ntff_profile_hook = lambda h: None
        import antenv
        antenv.axon_hooks = mod
        sys.modules["antenv.axon_hooks"] = mod
    except Exception:
        pass


def _run(x, cos, sin, W_qkv, W_proj, trace=False):
    from concourse.bass_utils import run_bass_kernel_spmd

    if trace:
        _install_ntff_hook()

    x = np.ascontiguousarray(x, dtype=np.float32)
    cos = np.ascontiguousarray(cos, dtype=np.float32)
    sin = np.ascontiguousarray(sin, dtype=np.float32)
    W_qkv = np.ascontiguousarray(W_qkv, dtype=np.float32)
    W_proj = np.ascontiguousarray(W_proj, dtype=np.float32)

    Bp, Tp, Cp = x.shape
    nc = _build(Bp, Tp, Cp, HPC, D)
    in_maps = _prep_in_maps(x, cos, sin, W_qkv, W_proj, N_CORES, HPC, D)
    res = run_bass_kernel_spmd(nc, in_maps, core_ids=list(range(N_CORES)),
                               trace=trace)
    acc = np.zeros((Bp * Tp, Cp), dtype=np.float32)
    for i in range(N_CORES):
        o = np.asarray(res.results[i]["out"], dtype=np.float32)
        acc += o.transpose(1, 0, 2).reshape(Bp * Tp, Cp)
    return acc.reshape(Bp, Tp, Cp), res


def kernel(x, cos, sin, W_qkv, W_proj):
    out, _ = _run(x, cos, sin, W_qkv, W_proj, trace=False)
    return out
